# revision 1
# baseline (speedup 1.0000x reference)
# Causal GQA self-attention on 8 TRN2 NeuronCores (Bass/Tile, SPMD).
#
# Sharding: core c -> (batch b = c//4, head-group g = c%4). Each core computes
# q-heads 4g..4g+3 with kv-head g for its batch (attention phase), then an
# 8-rank AllToAll re-shards attention outputs from head-split to token-split:
# core c does the output projection for token rows [256c, 256c+256) of BOTH
# batches with the full Wo. Outputs are pure per-core slices (host concat).
#
# All matmuls run as float32r (full-rate fp32 on the PE, ~1e-4 rel error).
# x is loaded in natural layout and transposed on-device via the PE transpose
# path (DRAM-side transposing DMAs decompose into 4-byte packets and are ~50x
# too slow).
import numpy as np

B, T, C = 2, 2048, 2048
H, KV, HD = 16, 4, 128
NCORES = 8
INV_SQRT_HD = 1.0 / float(np.sqrt(HD))
NEG = -1.0e30

_cache = {}


def _build(t_len, c_len):
    import concourse.bass as bass
    import concourse.mybir as mybir
    import concourse.tile as tile
    from concourse import bacc
    from concourse.masks import make_identity

    F32 = mybir.dt.float32
    F32R = mybir.dt.float32r
    AF = mybir.ActivationFunctionType
    MUL = mybir.AluOpType.mult
    ADD = mybir.AluOpType.add

    NT = t_len // 128          # token tiles
    NC_ = c_len // 128         # channel tiles
    NCH = t_len // 512         # 512-wide token chunks
    HL = 4                     # local q heads
    TS = t_len // 8            # per-core token slice for o_proj

    nc = bacc.Bacc("TRN2", target_bir_lowering=False, debug=False,
                   num_devices=NCORES)

    xT_ap = nc.dram_tensor("xT", [c_len, t_len], F32, kind="ExternalInput").ap()
    wq_ap = nc.dram_tensor("wq", [c_len, 512], F32, kind="ExternalInput").ap()
    wk_ap = nc.dram_tensor("wk", [c_len, 128], F32, kind="ExternalInput").ap()
    wv_ap = nc.dram_tensor("wv", [c_len, 128], F32, kind="ExternalInput").ap()
    wo_ap = nc.dram_tensor("wo", [2048, 2048], F32, kind="ExternalInput").ap()
    cos_ap = nc.dram_tensor("cosT", [128, t_len], F32, kind="ExternalInput").ap()
    sin_ap = nc.dram_tensor("sinTs", [128, t_len], F32, kind="ExternalInput").ap()
    msk_ap = nc.dram_tensor("masks", [4 * 128, 512], F32, kind="ExternalInput").ap()
    o_aps = [nc.dram_tensor(f"o{bb}", [2048, TS], F32, kind="ExternalOutput").ap()
             for bb in range(2)]
    a2in_a = nc.dram_tensor("a2in_a", [NCORES * 256, TS], F32).ap()
    a2out_a = nc.dram_tensor("a2out_a", [NCORES * 256, TS], F32).ap()
    a2in_b = nc.dram_tensor("a2in_b", [NCORES * 256, TS], F32).ap()
    a2out_b = nc.dram_tensor("a2out_b", [NCORES * 256, TS], F32).ap()

    with tile.TileContext(nc) as tc:
        with tc.tile_pool(name="const", bufs=1) as constp:
            idt = constp.tile([128, 128], F32)
            make_identity(nc, idt[:, :])
            ones_k = constp.tile([128, 1], F32)
            nc.vector.memset(ones_k[:, :], 1.0)
            ones_m = constp.tile([1, 128], F32)
            nc.vector.memset(ones_m[:, :], 1.0)

            JTA = [jt for jt in range(16) if jt % 4 < 2]
            JTB = [jt for jt in range(16) if jt % 4 >= 2]
            with tc.tile_pool(name="act", bufs=1) as pp:
                # persistent activations (freed before the o_proj phase)
                qT = [pp.tile([128, t_len], F32R, tag=f"qT{j}", name=f"qT{j}")
                      for j in range(HL)]
                kT = pp.tile([128, t_len], F32R)
                v_t = [pp.tile([128, 128], F32R, tag=f"v{tt}", name=f"v{tt}")
                       for tt in range(NT)]

                # ------------ phase 1+2: transpose x + projections + RoPE
                with (
                    tc.tile_pool(name="ph2", bufs=1) as ph2,
                    tc.tile_pool(name="ph2ps", bufs=2, space="PSUM") as ph2ps,
                ):
                    cosT = ph2.tile([128, t_len], F32)
                    nc.sync.dma_start(out=cosT[:, :], in_=cos_ap[:, :])
                    sinTs = ph2.tile([128, t_len], F32)
                    nc.sync.dma_start(out=sinTs[:, :], in_=sin_ap[:, :])
                    wq_sb = [ph2.tile([128, 512], F32R, tag=f"wq{ct}", name=f"wq{ct}")
                             for ct in range(NC_)]
                    wk_sb = [ph2.tile([128, 128], F32R, tag=f"wk{ct}", name=f"wk{ct}")
                             for ct in range(NC_)]
                    wv_sb = [ph2.tile([128, 128], F32R, tag=f"wv{ct}", name=f"wv{ct}")
                             for ct in range(NC_)]
                    xs0 = [ph2.tile([128, 512], F32R, tag=f"xs{ct}", name=f"xs0_{ct}",
                                    bufs=2)
                           for ct in range(NC_)]
                    for ct in range(NC_):
                        nc.sync.dma_start(out=xs0[ct][:, :],
                                          in_=xT_ap[ct*128:(ct+1)*128, 0:512].bitcast(F32R))
                    for ct in range(NC_):
                        nc.sync.dma_start(out=wq_sb[ct][:, :],
                                          in_=wq_ap[ct*128:(ct+1)*128, :].bitcast(F32R))
                        nc.sync.dma_start(out=wk_sb[ct][:, :],
                                          in_=wk_ap[ct*128:(ct+1)*128, :].bitcast(F32R))
                        nc.sync.dma_start(out=wv_sb[ct][:, :],
                                          in_=wv_ap[ct*128:(ct+1)*128, :].bitcast(F32R))
                    for ch in range(NCH):
                        sl = slice(ch * 512, (ch + 1) * 512)
                        if ch == 0:
                            xs = xs0
                        else:
                            xs = [ph2.tile([128, 512], F32R, tag=f"xs{ct}",
                                           name=f"xs{ct}", bufs=2)
                                  for ct in range(NC_)]
                            for ct in range(NC_):
                                nc.sync.dma_start(out=xs[ct][:, :],
                                                  in_=xT_ap[ct*128:(ct+1)*128, sl].bitcast(F32R))
                        # q heads + k: project, then RoPE
                        for u in range(HL + 1):
                            ps_a = ph2ps.tile([128, 512], F32, tag="acc", name="ps_a")
                            for ct in range(NC_):
                                w = wq_sb[ct][:, u*128:(u+1)*128] if u < HL else wk_sb[ct][:, :]
                                nc.tensor.matmul(out=ps_a[:, :], lhsT=w, rhs=xs[ct][:, :],
                                                 start=(ct == 0), stop=(ct == NC_ - 1))
                            raw = ph2.tile([128, 512], F32, tag="raw", bufs=2, name="raw")
                            nc.scalar.activation(raw[:, :], ps_a[:, :], AF.Copy)
                            sw = ph2.tile([128, 512], F32, tag="sw", bufs=1, name="sw")
                            nc.sync.dma_start(out=sw[0:64, :], in_=raw[64:128, :])
                            nc.sync.dma_start(out=sw[64:128, :], in_=raw[0:64, :])
                            t1 = ph2.tile([128, 512], F32, tag="t1", bufs=1, name="t1")
                            nc.vector.tensor_tensor(t1[:, :], sw[:, :], sinTs[:, sl], MUL)
                            t2 = ph2.tile([128, 512], F32, tag="t2", bufs=1, name="t2")
                            nc.vector.tensor_tensor(t2[:, :], raw[:, :], cosT[:, sl], MUL)
                            dst = qT[u][:, sl] if u < HL else kT[:, sl]
                            nc.vector.tensor_tensor(dst, t1[:, :], t2[:, :], ADD)
                        # v: project then transpose to token-major
                        ps_a = ph2ps.tile([128, 512], F32, tag="acc", name="ps_av")
                        for ct in range(NC_):
                            nc.tensor.matmul(out=ps_a[:, :], lhsT=wv_sb[ct][:, :],
                                             rhs=xs[ct][:, :],
                                             start=(ct == 0), stop=(ct == NC_ - 1))
                        vraw = ph2.tile([128, 512], F32, tag="vraw", bufs=1, name="vraw")
                        nc.scalar.activation(vraw[:, :], ps_a[:, :], AF.Copy)
                        for tt4 in range(4):
                            ps_tr = ph2ps.tile([128, 128], F32, tag="tr", name="ps_trv")
                            nc.tensor.transpose(ps_tr[:, :], vraw[:, tt4*128:(tt4+1)*128],
                                                idt[:, :])
                            nc.vector.tensor_copy(v_t[ch*4+tt4][:, :], ps_tr[:, :])

                # preload 12 of 16 Wo row-tiles during attention (gpsimd DMA
                # queue so they don't head-of-line-block the sync queue)
                JTPRE = JTA + [2, 6, 10, 14]
                JTSTR = [3, 7, 11, 15]
                woap = tc.alloc_tile_pool(name="woa", bufs=1)
                wo_a = {jt: woap.tile([128, 2048], F32R, tag=f"wo{jt}",
                                      name=f"wo{jt}") for jt in JTPRE}
                mskp = tc.alloc_tile_pool(name="mskp", bufs=1)
                masks = [mskp.tile([128, 512], F32, tag=f"msk{i}", name=f"msk{i}")
                         for i in range(4)]
                for i in range(4):
                    nc.sync.dma_start(out=masks[i][:, :],
                                      in_=msk_ap[i*128:(i+1)*128, :])
                for jt in JTPRE:
                    nc.gpsimd.dma_start(out=wo_a[jt][:, :],
                                        in_=wo_ap[jt*128:(jt+1)*128, :].bitcast(F32R))

                # ---------------- phase 3: attention (4 heads x NCH chunks)
                with (
                    tc.tile_pool(name="ph3", bufs=1) as ph3,
                    tc.tile_pool(name="ph3ps", bufs=2, space="PSUM") as ph3ps,
                ):
                    for h in range(HL):
                        for ch in range(NCH):
                            sl = slice(ch * 512, (ch + 1) * 512)
                            njt = 4 * ch + 4
                            att = []
                            ps_av = ph3ps.tile([128, 512], F32, tag="av", name="ps_av3")
                            ps_dn = ph3ps.tile([1, 512], F32, tag="den", name="ps_dn")
                            for jt in range(njt):
                                ps_s = ph3ps.tile([128, 512], F32, tag="s", name="ps_s")
                                nc.tensor.matmul(out=ps_s[:, :],
                                                 lhsT=kT[:, jt*128:(jt+1)*128],
                                                 rhs=qT[h][:, sl], start=True, stop=True)
                                a = ph3.tile([128, 512], F32R, tag=f"att{jt % 4}",
                                             bufs=2, name=f"att{jt}")
                                if jt >= 4 * ch:
                                    sm = ph3.tile([128, 512], F32, tag="sm", bufs=2,
                                                  name="sm")
                                    nc.vector.scalar_tensor_tensor(
                                        out=sm[:, :], in0=ps_s[:, :], scalar=INV_SQRT_HD,
                                        in1=masks[jt - 4 * ch][:, :], op0=MUL, op1=ADD)
                                    nc.scalar.activation(a[:, :], sm[:, :], AF.Exp)
                                else:
                                    nc.scalar.activation(a[:, :], ps_s[:, :], AF.Exp,
                                                         scale=INV_SQRT_HD)
                                att.append(a)
                                nc.tensor.matmul(out=ps_dn[:, :],
                                                 lhsT=ones_k[:, :].bitcast(F32R),
                                                 rhs=a[:, :], start=(jt == 0),
                                                 stop=(jt == njt - 1))
                                nc.tensor.matmul(out=ps_av[:, :], lhsT=v_t[jt][:, :],
                                                 rhs=a[:, :], start=(jt == 0),
                                                 stop=(jt == njt - 1))
                            dn = ph3.tile([1, 512], F32R, tag="dn", bufs=2, name="dn")
                            nc.vector.tensor_copy(dn[:, :], ps_dn[:, :])
                            ps_bc = ph3ps.tile([128, 512], F32, tag="bc", bufs=1, name="ps_bc")
                            nc.tensor.matmul(out=ps_bc[:, :],
                                             lhsT=ones_m[:, :].bitcast(F32R),
                                             rhs=dn[:, :], start=True, stop=True)
                            rec = ph3.tile([128, 512], F32, tag="rec", bufs=2, name="rec")
                            nc.vector.reciprocal_approx_fast(rec[:, :], ps_bc[:, :])
                            ao = ph3.tile([128, 512], F32, tag="ao", bufs=2, name="ao")
                            nc.vector.tensor_tensor(ao[:, :], ps_av[:, :], rec[:, :], MUL)
                            dst = a2in_a if h < 2 else a2in_b
                            hh = h if h < 2 else h - 2
                            for half in range(512 // TS):
                                tb = (512 // TS) * ch + half
                                nc.sync.dma_start(
                                    out=dst[tb*256 + hh*128: tb*256 + (hh+1)*128, :],
                                    in_=ao[:, half*TS:(half+1)*TS])
                        if h == 1:
                            nc.gpsimd.collective_compute(
                                "AllToAll", mybir.AluOpType.bypass,
                                replica_groups=[list(range(NCORES))],
                                ins=[a2in_a[:, :]], outs=[a2out_a[:, :]],
                            )

                mskp.release()

                # ---------------- A2A-b: heads 2-3
                nc.gpsimd.collective_compute(
                    "AllToAll", mybir.AluOpType.bypass,
                    replica_groups=[list(range(NCORES))],
                    ins=[a2in_b[:, :]], outs=[a2out_b[:, :]],
                )

                # -------- phase 4: o_proj (part-a preloaded, part-b streamed)
                with (
                    tc.tile_pool(name="ph4", bufs=1) as ph4,
                    tc.tile_pool(name="ph4ps", bufs=2, space="PSUM") as ph4ps,
                ):
                    rhs_t = {}
                    for jt in JTA:
                        t_ = ph4.tile([128, 2 * TS], F32R, tag=f"rhs{jt}",
                                      name=f"rhs{jt}")
                        rhs_t[jt] = t_
                        for bb in range(2):
                            row = (4 * bb + jt // 4) * 256 + (jt % 4) * 128
                            nc.sync.dma_start(out=t_[:, bb*TS:(bb+1)*TS],
                                              in_=a2out_a[row:row+128, :].bitcast(F32R))
                    for jt in JTB:
                        t_ = ph4.tile([128, 2 * TS], F32R, tag=f"rhs{jt}",
                                      name=f"rhs{jt}")
                        rhs_t[jt] = t_
                        for bb in range(2):
                            row = (4 * bb + jt // 4) * 256 + (jt % 4 - 2) * 128
                            nc.sync.dma_start(out=t_[:, bb*TS:(bb+1)*TS],
                                              in_=a2out_b[row:row+128, :].bitcast(F32R))
                    JTORD = JTPRE + JTSTR
                    for cc in range(16):
                        wob = {}
                        for jt in JTSTR:
                            w_ = ph4.tile([128, 128], F32R, tag=f"wob{jt}", bufs=2,
                                          name=f"wob{jt}")
                            wob[jt] = w_
                            nc.sync.dma_start(
                                out=w_[:, :],
                                in_=wo_ap[jt*128:(jt+1)*128,
                                          cc*128:(cc+1)*128].bitcast(F32R))
                        ps_o = ph4ps.tile([128, 2 * TS], F32, tag="o", name="ps_o")
                        for idx, jt in enumerate(JTORD):
                            w_ = (wo_a[jt][:, cc*128:(cc+1)*128] if jt in wo_a
                                  else wob[jt][:, :])
                            nc.tensor.matmul(out=ps_o[:, :], lhsT=w_,
                                             rhs=rhs_t[jt][:, :],
                                             start=(idx == 0), stop=(idx == 15))
                        osb = ph4.tile([128, 2 * TS], F32, tag="osb", bufs=2,
                                       name="osb")
                        nc.scalar.activation(osb[:, :], ps_o[:, :], AF.Copy)
                        for bb in range(2):
                            nc.sync.dma_start(out=o_aps[bb][cc*128:(cc+1)*128, :],
                                              in_=osb[:, bb*TS:(bb+1)*TS])
                woap.release()

    nc.compile()
    return nc


def _prep_inputs(x, cos, sin, Wq, Wk, Wv, Wo):
    x = np.ascontiguousarray(np.asarray(x), dtype=np.float32)
    cos = np.asarray(cos, dtype=np.float32)
    sin = np.asarray(sin, dtype=np.float32)
    Wq = np.ascontiguousarray(np.asarray(Wq), dtype=np.float32)
    Wk = np.ascontiguousarray(np.asarray(Wk), dtype=np.float32)
    Wv = np.ascontiguousarray(np.asarray(Wv), dtype=np.float32)
    Wo = np.ascontiguousarray(np.asarray(Wo), dtype=np.float32)

    t_len = x.shape[1]
    cosT = np.ascontiguousarray(cos.T)                       # [128, T]
    sinT = np.ascontiguousarray(sin.T)
    sinTs = sinT.copy()
    sinTs[0:64, :] *= -1.0                                   # signed swap-half

    masks = np.zeros((4, 128, 512), dtype=np.float32)
    tk = np.arange(128)[:, None]
    tq = np.arange(512)[None, :]
    for jd in range(4):
        masks[jd] = np.where(128 * jd + tk <= tq, 0.0, NEG)
    masks = masks.reshape(4 * 128, 512)

    in_maps = []
    for c in range(NCORES):
        b, g = c // 4, c % 4
        xb = x[b] if x.ndim == 3 else x
        in_maps.append({
            "xT": np.ascontiguousarray(xb.T),
            "wq": np.ascontiguousarray(Wq[:, 512*g:512*(g+1)]),
            "wk": np.ascontiguousarray(Wk[:, 128*g:128*(g+1)]),
            "wv": np.ascontiguousarray(Wv[:, 128*g:128*(g+1)]),
            "wo": Wo,
            "cosT": cosT,
            "sinTs": np.ascontiguousarray(sinTs),
            "masks": masks,
        })
    return in_maps, t_len


def kernel(x, cos, sin, Wq, Wk, Wv, Wo):
    from concourse.bass_utils import run_bass_kernel_spmd

    in_maps, t_len = _prep_inputs(x, cos, sin, Wq, Wk, Wv, Wo)
    c_len = in_maps[0]["xT"].shape[0]
    key = (t_len, c_len)
    if key not in _cache:
        _cache[key] = _build(t_len, c_len)
    nc = _cache[key]

    res = run_bass_kernel_spmd(nc, in_maps, core_ids=list(range(NCORES)))
    ts = t_len // 8
    out = np.empty((2, t_len, 2048), dtype=np.float32)
    for c in range(NCORES):
        out[0, ts*c:ts*(c+1), :] = res.results[c]["o0"].T
        out[1, ts*c:ts*(c+1), :] = res.results[c]["o1"].T
    return out

